# revision 19
# baseline (speedup 1.0000x reference)
"""AdaptiveMixGNNLayer distributed Trainium2 kernel (8 NeuronCores).

out = relu(alpha * (S_LP @ x) @ W_LP^T + (1-alpha) * (S_HP @ x) @ W_HP^T + bias)

Strategy:
  - Shard destination rows across 8 cores (6250 rows each); each core owns the
    edges whose destination row falls in its range (rows are sorted, so this is
    a contiguous slice of each edge array).
  - x (the gather table, cast to bf16) is replicated into every core's HBM;
    gathers are core-local HBM->SBUF via the gpsimd dma_gather instruction.
    Gather calls are spread across all 4 SWDGE queues: each queue's descriptor
    generation runs on a different Q7 core pair (dma_gather.cpp dispatches on
    cpu_id/2 == queue_num), which parallelizes the per-index descriptor
    generation (the dominant cost) ~3-4x.
  - dma_gather indices are int16 (< 32768), so each block's edges are split by
    source column: "lo" edges (col < SPLIT) gather from table base 0 with
    idx=col, "hi" edges (col >= SPLIT) gather from an AP offset of HI_OFF rows
    with idx = col - HI_OFF.
  - Per core, destination rows are grouped into blocks of 128 rows. Per
    (set, block) the lo/hi edges are padded to T_lo/T_hi tiles of 128 edges
    (pad edges have val=0). For each tile the aggregation matrix
    A[e, r] = val[e] * (row_rel[e] == r) is PRECOMPUTED ON THE HOST in bf16
    and bulk-streamed from HBM (no on-chip build), and TensorE accumulates
    aggT[f, r] += G^T @ A into PSUM over the block's tiles.
  - Block epilogue: aggT -> SBUF f32, psum2 = W_LP^T.T @ aggT_lp +
    W_HP^T.T @ aggT_hp (accumulated in one PSUM bank; alpha folded into vals
    on host), then out = relu(psum2 + bias) on ScalarE; DMA block to DRAM.
  - Host unshards [NBLK, 128o, 128r] per-core outputs back to [N, 128].
"""

import os
import numpy as np

N_NODES = 50000
N_EDGES = 640000
D = 128
NCORES = 8
ROWS_PER_CORE = N_NODES // NCORES  # 6250
PACK_CAP = 640  # per-(set,half) edge cap per block for greedy packing
HI_OFF = N_NODES - 32768  # 17232
SPLIT = 25000  # lo/hi column boundary; must be in [HI_OFF, 32768)
GATHER_GROUP = 8  # blocks per dma_gather call / A-stream chunk

_COMPILED = {}


def _plan_blocks(lp_rows, lp_cols, hp_rows, hp_cols, cap):
    """Greedy-pack destination rows into blocks of <=128 rows such that each
    block's edge count stays <= cap in each of the 4 (set, col-half) groups.
    All cores are padded to the same block count by splitting the largest
    blocks.  Returns per-core lists of (r_start, r_end) relative to the core.
    """
    def per_row(rows, cols):
        lo = np.bincount(rows[cols < SPLIT], minlength=N_NODES)
        hi = np.bincount(rows[cols >= SPLIT], minlength=N_NODES)
        return lo, hi

    lp_lo, lp_hi = per_row(np.asarray(lp_rows), np.asarray(lp_cols))
    hp_lo, hp_hi = per_row(np.asarray(hp_rows), np.asarray(hp_cols))
    grp = np.stack([lp_lo, lp_hi, hp_lo, hp_hi], axis=1)  # [N, 4]

    plans = []
    for c in range(NCORES):
        r0 = c * ROWS_PER_CORE
        blocks = []
        start = 0
        cnt = np.zeros(4, np.int64)
        for r in range(ROWS_PER_CORE):
            add = grp[r0 + r]
            if (r - start) >= 128 or np.any(cnt + add > cap):
                blocks.append((start, r))
                start = r
                cnt = add.copy()
            else:
                cnt += add
        blocks.append((start, ROWS_PER_CORE))
        plans.append(blocks)

    nblk = max(len(b) for b in plans)
    for c in range(NCORES):
        blocks = plans[c]
        while len(blocks) < nblk:
            # split the block with the most rows
            widths = [e - st for st, e in blocks]
            i = int(np.argmax(widths))
            st, e = blocks[i]
            mid = st + (e - st) // 2
            blocks[i:i + 1] = [(st, mid), (mid, e)]
        plans[c] = blocks
    return plans, nblk


def _prep_set(rows, cols, vals, plans, nblk):
    """Partition one edge set by destination-row block / column half.

    Returns (idx_lo, idx_hi, amat, T_lo, T_hi):
      idx_lo: [NCORES, 128, nblk*T_lo*8] int16 gather indices, wrapped mod 16
              (partition p row = p % 16) in flat tile order, replicated 8x down
      idx_hi: same for the hi half
      amat:   [NCORES, 128, nblk*T*128] float32 aggregation matrices;
              amat[c, e, (b*T+t)*128 + r] = val * (row_rel == r) for the edge
              at lane e of tile t of block b (lo tiles t < T_lo, then hi)
    """
    rows = np.asarray(rows)
    cols = np.asarray(cols)
    vals = np.asarray(vals)

    core_bounds = np.searchsorted(rows, np.arange(NCORES + 1) * ROWS_PER_CORE)
    per_core = []
    n_lo_max = 0
    n_hi_max = 0
    for c in range(NCORES):
        e0 = core_bounds[c]
        r = rows[core_bounds[c]:core_bounds[c + 1]] - c * ROWS_PER_CORE
        bounds = [st for st, _ in plans[c]] + [ROWS_PER_CORE]
        bb = np.searchsorted(r, bounds)
        blocks = []
        for b in range(nblk):
            s, e = e0 + bb[b], e0 + bb[b + 1]
            bcols = cols[s:e]
            bvals = vals[s:e]
            brow = (rows[s:e] - c * ROWS_PER_CORE - plans[c][b][0]).astype(np.int64)
            lo = bcols < SPLIT
            hi = ~lo
            blocks.append((bcols[lo], bvals[lo], brow[lo],
                           bcols[hi], bvals[hi], brow[hi]))
            n_lo_max = max(n_lo_max, int(lo.sum()))
            n_hi_max = max(n_hi_max, int(hi.sum()))
        per_core.append(blocks)

    T_lo = max(1, (n_lo_max + 127) // 128)
    T_hi = max(1, (n_hi_max + 127) // 128)
    T = T_lo + T_hi

    flat_lo = np.zeros((NCORES, nblk * T_lo * 128), dtype=np.int16)
    flat_hi = np.zeros((NCORES, nblk * T_hi * 128), dtype=np.int16)
    amat = np.zeros((NCORES, nblk * T, 128, 128), dtype=np.float32)
    rr = np.zeros((NCORES, 128, nblk * T), dtype=np.float32)
    val = np.zeros((NCORES, 128, nblk * T), dtype=np.float32)

    for c in range(NCORES):
        for b in range(nblk):
            lc, lv, lr, hc, hv, hr = per_core[c][b]
            n = len(lc)
            if n:
                j = np.arange(n)
                flat_lo[c, b * T_lo * 128 + j] = lc.astype(np.int16)
                amat[c, b * T + j // 128, j % 128, lr] = lv
                rr[c, j % 128, b * T + j // 128] = lr
                val[c, j % 128, b * T + j // 128] = lv
            n = len(hc)
            if n:
                j = np.arange(n)
                flat_hi[c, b * T_hi * 128 + j] = (hc - HI_OFF).astype(np.int16)
                amat[c, b * T + T_lo + j // 128, j % 128, hr] = hv
                rr[c, j % 128, b * T + T_lo + j // 128] = hr
                val[c, j % 128, b * T + T_lo + j // 128] = hv

    def wrap(flat):
        L = flat.shape[1]
        w = flat.reshape(NCORES, L // 16, 16).transpose(0, 2, 1)
        return np.ascontiguousarray(np.tile(w, (1, 8, 1)))

    amat = np.ascontiguousarray(
        amat.transpose(0, 2, 1, 3).reshape(NCORES, 128, nblk * T * 128))
    return wrap(flat_lo), wrap(flat_hi), amat, rr, val, T_lo, T_hi


def _build(nblk, T_lp_lo, T_lp_hi, T_hp_lo, T_hp_hi, abuild):
    import concourse.bacc as bacc
    import concourse.mybir as mybir
    import concourse.tile as tile

    f32 = mybir.dt.float32
    bf16 = mybir.dt.bfloat16
    i16 = mybir.dt.int16

    nqueues = int(os.environ.get("KERNEL_NQUEUES", "4"))
    nc = bacc.Bacc("TRN2", target_bir_lowering=False, num_swdge_queues=4)

    x_t = nc.dram_tensor("xbf", [N_NODES, D], bf16, kind="ExternalInput")
    iota_t = (nc.dram_tensor("iota", [128, 128], bf16, kind="ExternalInput")
              if abuild else None)
    wlpT_t = nc.dram_tensor("wlpT", [D, D], f32, kind="ExternalInput")
    whpT_t = nc.dram_tensor("whpT", [D, D], f32, kind="ExternalInput")
    bias_t = nc.dram_tensor("bias", [128, 1], f32, kind="ExternalInput")
    out_t = nc.dram_tensor("out", [nblk, 128, 128], f32, kind="ExternalOutput")

    set_T = {"lp": (T_lp_lo, T_lp_hi), "hp": (T_hp_lo, T_hp_hi)}
    dram = {}
    for s in ("lp", "hp"):
        T_lo, T_hi = set_T[s]
        T = T_lo + T_hi
        dram[s] = {
            "idx_lo": nc.dram_tensor(f"idx_lo_{s}", [128, nblk * T_lo * 8], i16,
                                     kind="ExternalInput"),
            "idx_hi": nc.dram_tensor(f"idx_hi_{s}", [128, nblk * T_hi * 8], i16,
                                     kind="ExternalInput"),
        }
        if abuild:
            dram[s]["rr"] = nc.dram_tensor(f"rr_{s}", [128, nblk * T], f32,
                                           kind="ExternalInput")
            dram[s]["val"] = nc.dram_tensor(f"val_{s}", [128, nblk * T], f32,
                                            kind="ExternalInput")
        else:
            dram[s]["amat"] = nc.dram_tensor(f"amat_{s}", [128, nblk * T * 128],
                                             bf16, kind="ExternalInput")

    # taper: full-size groups first, small groups at the end to shrink the
    # pipeline drain tail
    taper = [3, 2, 2, 1]
    sizes = []
    rem = nblk
    while rem > sum(taper):
        take = min(GATHER_GROUP, rem - sum(taper))
        sizes.append(take)
        rem -= take
    for t in taper:
        if rem <= 0:
            break
        take = min(t, rem)
        sizes.append(take)
        rem -= take
    groups = []
    b0 = 0
    for nb in sizes:
        groups.append((b0, nb))
        b0 += nb
    ngroups = int(os.environ.get("KERNEL_NGROUPS", "0"))
    if ngroups:
        groups = groups[:ngroups]

    qrr = [0]  # round-robin SWDGE queue assignment

    with tile.TileContext(nc) as tc:
        with (
            tc.tile_pool(name="const", bufs=1) as cpool,
            tc.tile_pool(name="gbuf", bufs=2) as gpool,
            tc.tile_pool(name="abuf", bufs=4 if abuild else 2) as apool,
            tc.tile_pool(name="cagg", bufs=4) as caggpool,
            tc.tile_pool(name="osb", bufs=2) as opool,
            tc.tile_pool(name="psagg", bufs=2, space="PSUM") as psagg,
            tc.tile_pool(name="ps2", bufs=2, space="PSUM") as ps2,
        ):
            consts = {}
            for s in ("lp", "hp"):
                consts[s] = {
                    k: cpool.tile_from(t[:], name=f"{k}_{s}")
                    for k, t in dram[s].items()
                    if k != "amat"
                }
            iota = cpool.tile_from(iota_t[:], name="iota") if abuild else None
            wlpT = cpool.tile_from(wlpT_t[:], name="wlpT")
            whpT = cpool.tile_from(whpT_t[:], name="whpT")
            bias = cpool.tile_from(bias_t[:], name="bias")

            for b0, nb in groups:
                gtiles = {}
                atiles = {}
                for s in ("lp", "hp"):
                    T_lo, T_hi = set_T[s]
                    T = T_lo + T_hi
                    for half, T_h, src in (
                        ("lo", T_lo, x_t[0:32768, :]),
                        ("hi", T_hi, x_t[HI_OFF : HI_OFF + 32768, :]),
                    ):
                        g = gpool.tile([128, GATHER_GROUP * T_h, 128], bf16,
                                       tag=f"g_{s}_{half}")
                        n_idx = nb * T_h * 128
                        nc.gpsimd.dma_gather(
                            g[:, : nb * T_h, :],
                            src,
                            consts[s][f"idx_{half}"][
                                :, b0 * T_h * 8 : (b0 + nb) * T_h * 8
                            ],
                            n_idx,
                            n_idx,
                            D,
                            single_packet=False,
                            queue_num=qrr[0] % nqueues,
                        )
                        qrr[0] += 1
                        gtiles[(s, half)] = g
                    if not abuild:
                        a = apool.tile([128, GATHER_GROUP * T * 128], bf16,
                                       tag=f"a_{s}")
                        nc.sync.dma_start(
                            a[:, : nb * T * 128],
                            dram[s]["amat"][:, b0 * T * 128 : (b0 + nb) * T * 128],
                        )
                        atiles[s] = a

                for bi in range(nb):
                    b = b0 + bi
                    caggs = {}
                    for s in ("lp", "hp"):
                        T_lo, T_hi = set_T[s]
                        T = T_lo + T_hi
                        aggT = psagg.tile([128, 128], f32, tag=f"aggT_{s}")
                        for t in range(T):
                            if t < T_lo:
                                gsl = gtiles[(s, "lo")][:, bi * T_lo + t, :]
                            else:
                                gsl = gtiles[(s, "hi")][:, bi * T_hi + t - T_lo, :]
                            if abuild:
                                asl_t = apool.tile([128, 128], bf16, tag="A")
                                sl = b * T + t
                                nc.vector.tensor_scalar(
                                    asl_t[:],
                                    iota[:],
                                    consts[s]["rr"][:, sl : sl + 1],
                                    consts[s]["val"][:, sl : sl + 1],
                                    mybir.AluOpType.is_equal,
                                    mybir.AluOpType.mult,
                                )
                                asl = asl_t[:]
                            else:
                                asl = atiles[s][
                                    :, (bi * T + t) * 128 : (bi * T + t + 1) * 128
                                ]
                            nc.tensor.matmul(
                                aggT[:],
                                gsl,
                                asl,
                                start=(t == 0),
                                stop=(t == T - 1),
                            )
                        cagg = caggpool.tile([128, 128], f32, tag=f"cagg_{s}")
                        nc.scalar.copy(cagg[:], aggT[:])
                        caggs[s] = cagg

                    psum2 = ps2.tile([128, 128], f32, tag="psum2")
                    nc.tensor.matmul(psum2[:], wlpT[:], caggs["lp"][:],
                                     start=True, stop=False)
                    nc.tensor.matmul(psum2[:], whpT[:], caggs["hp"][:],
                                     start=False, stop=True)
                    osb = opool.tile([128, 128], f32, tag="osb")
                    nc.scalar.activation(
                        osb[:], psum2[:], mybir.ActivationFunctionType.Relu,
                        bias=bias[:, 0:1],
                    )
                    nc.scalar.dma_start(out_t[b, :, :], osb[:])

    nc.compile()
    return nc


def kernel(x, lp_rows, lp_cols, lp_vals, hp_rows, hp_cols, hp_vals,
           W_LP, W_HP, bias, alpha_raw):
    import ml_dtypes
    from concourse.bass_utils import run_bass_kernel_spmd

    x = np.asarray(x, dtype=np.float32)
    alpha = 1.0 / (1.0 + np.exp(-float(np.asarray(alpha_raw).reshape(-1)[0])))

    plans, nblk = _plan_blocks(lp_rows, lp_cols, hp_rows, hp_cols, PACK_CAP)
    lp = _prep_set(lp_rows, lp_cols,
                   np.asarray(lp_vals, np.float32) * np.float32(alpha),
                   plans, nblk)
    hp = _prep_set(hp_rows, hp_cols,
                   np.asarray(hp_vals, np.float32) * np.float32(1.0 - alpha),
                   plans, nblk)
    idx_lo_lp, idx_hi_lp, amat_lp, rr_lp, val_lp, T_lp_lo, T_lp_hi = lp
    idx_lo_hp, idx_hi_hp, amat_hp, rr_hp, val_hp, T_hp_lo, T_hp_hi = hp
    abuild = os.environ.get("KERNEL_ABUILD", "stream") == "dve"
    key = (nblk, T_lp_lo, T_lp_hi, T_hp_lo, T_hp_hi, abuild)
    if key not in _COMPILED:
        _COMPILED[key] = _build(*key)
    nc = _COMPILED[key]

    bf = ml_dtypes.bfloat16
    xbf = np.ascontiguousarray(x.astype(bf))
    wlpT = np.ascontiguousarray(np.asarray(W_LP, np.float32).T)  # [d, o]
    whpT = np.ascontiguousarray(np.asarray(W_HP, np.float32).T)
    bias_col = np.ascontiguousarray(np.asarray(bias, np.float32).reshape(128, 1))

    iota_np = np.ascontiguousarray(
        np.tile(np.arange(128, dtype=np.float32)[None, :], (128, 1)).astype(bf))
    in_maps = []
    for c in range(NCORES):
        m = {
            "xbf": xbf,
            "idx_lo_lp": idx_lo_lp[c], "idx_hi_lp": idx_hi_lp[c],
            "idx_lo_hp": idx_lo_hp[c], "idx_hi_hp": idx_hi_hp[c],
            "wlpT": wlpT, "whpT": whpT, "bias": bias_col,
        }
        if abuild:
            m["iota"] = iota_np
            m["rr_lp"] = rr_lp[c]
            m["val_lp"] = val_lp[c]
            m["rr_hp"] = rr_hp[c]
            m["val_hp"] = val_hp[c]
        else:
            m["amat_lp"] = amat_lp[c].astype(bf)
            m["amat_hp"] = amat_hp[c].astype(bf)
        in_maps.append(m)

    trace = bool(int(os.environ.get("KERNEL_TRACE", "0")))
    res = run_bass_kernel_spmd(nc, in_maps, list(range(NCORES)), trace=trace)
    kernel.last_result = res

    out = np.empty((N_NODES, D), dtype=np.float32)
    for c in range(NCORES):
        oc = res.results[c]["out"]  # [nblk, 128o, 128r]
        base = c * ROWS_PER_CORE
        for b, (r0, r1) in enumerate(plans[c]):
            out[base + r0 : base + r1, :] = oc[b, :, : r1 - r0].T
    return out


# revision 20
# speedup vs baseline: 1.1948x; 1.1948x over previous
"""AdaptiveMixGNNLayer distributed Trainium2 kernel (8 NeuronCores).

out = relu(alpha * (S_LP @ x) @ W_LP^T + (1-alpha) * (S_HP @ x) @ W_HP^T + bias)

Strategy:
  - Shard destination rows across 8 cores (6250 rows each); each core owns the
    edges whose destination row falls in its range (rows are sorted, so this is
    a contiguous slice of each edge array).
  - x (the gather table, cast to bf16) is replicated into every core's HBM;
    gathers are core-local HBM->SBUF via the gpsimd dma_gather instruction.
    Gather calls are spread across all 4 SWDGE queues: each queue's descriptor
    generation runs on a different Q7 core pair (dma_gather.cpp dispatches on
    cpu_id/2 == queue_num), which parallelizes the per-index descriptor
    generation (the dominant cost) ~3-4x.
  - dma_gather indices are int16 (< 32768), so each block's edges are split by
    source column: "lo" edges (col < SPLIT) gather from table base 0 with
    idx=col, "hi" edges (col >= SPLIT) gather from an AP offset of HI_OFF rows
    with idx = col - HI_OFF.
  - Per core, destination rows are grouped into blocks of 128 rows. Per
    (set, block) the lo/hi edges are padded to T_lo/T_hi tiles of 128 edges
    (pad edges have val=0). For each tile the aggregation matrix
    A[e, r] = val[e] * (row_rel[e] == r) is PRECOMPUTED ON THE HOST in bf16
    and bulk-streamed from HBM (no on-chip build), and TensorE accumulates
    aggT[f, r] += G^T @ A into PSUM over the block's tiles.
  - Block epilogue: aggT -> SBUF f32, psum2 = W_LP^T.T @ aggT_lp +
    W_HP^T.T @ aggT_hp (accumulated in one PSUM bank; alpha folded into vals
    on host), then out = relu(psum2 + bias) on ScalarE; DMA block to DRAM.
  - Host unshards [NBLK, 128o, 128r] per-core outputs back to [N, 128].
"""

import os
import numpy as np

N_NODES = 50000
N_EDGES = 640000
D = 128
NCORES = 8
ROWS_PER_CORE = N_NODES // NCORES  # 6250
PACK_CAP = 640  # per-(set,half) edge cap per block for greedy packing
HI_OFF = N_NODES - 32768  # 17232
SPLIT = 25000  # lo/hi column boundary; must be in [HI_OFF, 32768)
GATHER_GROUP = 6  # blocks per dma_gather call / A-stream chunk

_COMPILED = {}


def _plan_blocks(lp_rows, lp_cols, hp_rows, hp_cols, cap):
    """Greedy-pack destination rows into blocks of <=128 rows such that each
    block's edge count stays <= cap in each of the 4 (set, col-half) groups.
    All cores are padded to the same block count by splitting the largest
    blocks.  Returns per-core lists of (r_start, r_end) relative to the core.
    """
    def per_row(rows, cols):
        lo = np.bincount(rows[cols < SPLIT], minlength=N_NODES)
        hi = np.bincount(rows[cols >= SPLIT], minlength=N_NODES)
        return lo, hi

    lp_lo, lp_hi = per_row(np.asarray(lp_rows), np.asarray(lp_cols))
    hp_lo, hp_hi = per_row(np.asarray(hp_rows), np.asarray(hp_cols))
    grp = np.stack([lp_lo, lp_hi, hp_lo, hp_hi], axis=1)  # [N, 4]

    plans = []
    for c in range(NCORES):
        r0 = c * ROWS_PER_CORE
        blocks = []
        start = 0
        cnt = np.zeros(4, np.int64)
        for r in range(ROWS_PER_CORE):
            add = grp[r0 + r]
            if (r - start) >= 128 or np.any(cnt + add > cap):
                blocks.append((start, r))
                start = r
                cnt = add.copy()
            else:
                cnt += add
        blocks.append((start, ROWS_PER_CORE))
        plans.append(blocks)

    nblk = max(len(b) for b in plans)
    for c in range(NCORES):
        blocks = plans[c]
        while len(blocks) < nblk:
            # split the block with the most rows
            widths = [e - st for st, e in blocks]
            i = int(np.argmax(widths))
            st, e = blocks[i]
            mid = st + (e - st) // 2
            blocks[i:i + 1] = [(st, mid), (mid, e)]
        plans[c] = blocks
    return plans, nblk


def _prep_set(rows, cols, vals, plans, nblk):
    """Partition one edge set by destination-row block / column half.

    Returns (idx_lo, idx_hi, amat, T_lo, T_hi):
      idx_lo: [NCORES, 128, nblk*T_lo*8] int16 gather indices, wrapped mod 16
              (partition p row = p % 16) in flat tile order, replicated 8x down
      idx_hi: same for the hi half
      amat:   [NCORES, 128, nblk*T*128] float32 aggregation matrices;
              amat[c, e, (b*T+t)*128 + r] = val * (row_rel == r) for the edge
              at lane e of tile t of block b (lo tiles t < T_lo, then hi)
    """
    rows = np.asarray(rows)
    cols = np.asarray(cols)
    vals = np.asarray(vals)

    core_bounds = np.searchsorted(rows, np.arange(NCORES + 1) * ROWS_PER_CORE)
    per_core = []
    n_lo_max = 0
    n_hi_max = 0
    for c in range(NCORES):
        e0 = core_bounds[c]
        r = rows[core_bounds[c]:core_bounds[c + 1]] - c * ROWS_PER_CORE
        bounds = [st for st, _ in plans[c]] + [ROWS_PER_CORE]
        bb = np.searchsorted(r, bounds)
        blocks = []
        for b in range(nblk):
            s, e = e0 + bb[b], e0 + bb[b + 1]
            bcols = cols[s:e]
            bvals = vals[s:e]
            brow = (rows[s:e] - c * ROWS_PER_CORE - plans[c][b][0]).astype(np.int64)
            lo = bcols < SPLIT
            hi = ~lo
            blocks.append((bcols[lo], bvals[lo], brow[lo],
                           bcols[hi], bvals[hi], brow[hi]))
            n_lo_max = max(n_lo_max, int(lo.sum()))
            n_hi_max = max(n_hi_max, int(hi.sum()))
        per_core.append(blocks)

    T_lo = max(1, (n_lo_max + 127) // 128)
    T_hi = max(1, (n_hi_max + 127) // 128)
    T = T_lo + T_hi

    flat_lo = np.zeros((NCORES, nblk * T_lo * 128), dtype=np.int16)
    flat_hi = np.zeros((NCORES, nblk * T_hi * 128), dtype=np.int16)
    amat = np.zeros((NCORES, nblk * T, 128, 128), dtype=np.float32)
    rr = np.zeros((NCORES, 128, nblk * T), dtype=np.float32)
    val = np.zeros((NCORES, 128, nblk * T), dtype=np.float32)

    for c in range(NCORES):
        for b in range(nblk):
            lc, lv, lr, hc, hv, hr = per_core[c][b]
            n = len(lc)
            if n:
                j = np.arange(n)
                flat_lo[c, b * T_lo * 128 + j] = lc.astype(np.int16)
                amat[c, b * T + j // 128, j % 128, lr] = lv
                rr[c, j % 128, b * T + j // 128] = lr
                val[c, j % 128, b * T + j // 128] = lv
            n = len(hc)
            if n:
                j = np.arange(n)
                flat_hi[c, b * T_hi * 128 + j] = (hc - HI_OFF).astype(np.int16)
                amat[c, b * T + T_lo + j // 128, j % 128, hr] = hv
                rr[c, j % 128, b * T + T_lo + j // 128] = hr
                val[c, j % 128, b * T + T_lo + j // 128] = hv

    def wrap(flat):
        L = flat.shape[1]
        w = flat.reshape(NCORES, L // 16, 16).transpose(0, 2, 1)
        return np.ascontiguousarray(np.tile(w, (1, 8, 1)))

    amat = np.ascontiguousarray(
        amat.transpose(0, 2, 1, 3).reshape(NCORES, 128, nblk * T * 128))
    return wrap(flat_lo), wrap(flat_hi), amat, rr, val, T_lo, T_hi


def _build(nblk, T_lp_lo, T_lp_hi, T_hp_lo, T_hp_hi, abuild):
    import concourse.bacc as bacc
    import concourse.mybir as mybir
    import concourse.tile as tile

    f32 = mybir.dt.float32
    bf16 = mybir.dt.bfloat16
    i16 = mybir.dt.int16

    nqueues = int(os.environ.get("KERNEL_NQUEUES", "4"))
    nc = bacc.Bacc("TRN2", target_bir_lowering=False, num_swdge_queues=4)

    x_t = nc.dram_tensor("xbf", [N_NODES, D], bf16, kind="ExternalInput")
    iota_t = (nc.dram_tensor("iota", [128, 128], bf16, kind="ExternalInput")
              if abuild else None)
    wlpT_t = nc.dram_tensor("wlpT", [D, D], f32, kind="ExternalInput")
    whpT_t = nc.dram_tensor("whpT", [D, D], f32, kind="ExternalInput")
    bias_t = nc.dram_tensor("bias", [128, 1], f32, kind="ExternalInput")
    out_t = nc.dram_tensor("out", [nblk, 128, 128], f32, kind="ExternalOutput")

    set_T = {"lp": (T_lp_lo, T_lp_hi), "hp": (T_hp_lo, T_hp_hi)}
    dram = {}
    for s in ("lp", "hp"):
        T_lo, T_hi = set_T[s]
        T = T_lo + T_hi
        dram[s] = {
            "idx_lo": nc.dram_tensor(f"idx_lo_{s}", [128, nblk * T_lo * 8], i16,
                                     kind="ExternalInput"),
            "idx_hi": nc.dram_tensor(f"idx_hi_{s}", [128, nblk * T_hi * 8], i16,
                                     kind="ExternalInput"),
        }
        if abuild:
            dram[s]["rr"] = nc.dram_tensor(f"rr_{s}", [128, nblk * T], f32,
                                           kind="ExternalInput")
            dram[s]["val"] = nc.dram_tensor(f"val_{s}", [128, nblk * T], f32,
                                            kind="ExternalInput")
        else:
            dram[s]["amat"] = nc.dram_tensor(f"amat_{s}", [128, nblk * T * 128],
                                             bf16, kind="ExternalInput")

    # taper: full-size groups first, small groups at the end to shrink the
    # pipeline drain tail
    taper = [3, 2, 2, 1]
    sizes = []
    rem = nblk
    while rem > sum(taper):
        take = min(GATHER_GROUP, rem - sum(taper))
        sizes.append(take)
        rem -= take
    for t in taper:
        if rem <= 0:
            break
        take = min(t, rem)
        sizes.append(take)
        rem -= take
    groups = []
    b0 = 0
    for nb in sizes:
        groups.append((b0, nb))
        b0 += nb
    ngroups = int(os.environ.get("KERNEL_NGROUPS", "0"))
    if ngroups:
        groups = groups[:ngroups]

    qrr = [0]  # round-robin SWDGE queue assignment

    with tile.TileContext(nc) as tc:
        with (
            tc.tile_pool(name="const", bufs=1) as cpool,
            tc.tile_pool(name="gbuf", bufs=3) as gpool,
            tc.tile_pool(name="abuf", bufs=4 if abuild else 2) as apool,
            tc.tile_pool(name="cagg", bufs=4) as caggpool,
            tc.tile_pool(name="osb", bufs=2) as opool,
            tc.tile_pool(name="psagg", bufs=2, space="PSUM") as psagg,
            tc.tile_pool(name="ps2", bufs=2, space="PSUM") as ps2,
        ):
            consts = {}
            for s in ("lp", "hp"):
                consts[s] = {
                    k: cpool.tile_from(t[:], name=f"{k}_{s}")
                    for k, t in dram[s].items()
                    if k != "amat"
                }
            iota = cpool.tile_from(iota_t[:], name="iota") if abuild else None
            wlpT = cpool.tile_from(wlpT_t[:], name="wlpT")
            whpT = cpool.tile_from(whpT_t[:], name="whpT")
            bias = cpool.tile_from(bias_t[:], name="bias")

            for b0, nb in groups:
                gtiles = {}
                atiles = {}
                for s in ("lp", "hp"):
                    T_lo, T_hi = set_T[s]
                    T = T_lo + T_hi
                    for half, T_h, src in (
                        ("lo", T_lo, x_t[0:32768, :]),
                        ("hi", T_hi, x_t[HI_OFF : HI_OFF + 32768, :]),
                    ):
                        g = gpool.tile([128, GATHER_GROUP * T_h, 128], bf16,
                                       tag=f"g_{s}_{half}")
                        n_idx = nb * T_h * 128
                        nc.gpsimd.dma_gather(
                            g[:, : nb * T_h, :],
                            src,
                            consts[s][f"idx_{half}"][
                                :, b0 * T_h * 8 : (b0 + nb) * T_h * 8
                            ],
                            n_idx,
                            n_idx,
                            D,
                            single_packet=False,
                            queue_num=qrr[0] % nqueues,
                        )
                        qrr[0] += 1
                        gtiles[(s, half)] = g
                    if not abuild:
                        a = apool.tile([128, GATHER_GROUP * T * 128], bf16,
                                       tag=f"a_{s}")
                        nc.sync.dma_start(
                            a[:, : nb * T * 128],
                            dram[s]["amat"][:, b0 * T * 128 : (b0 + nb) * T * 128],
                        )
                        atiles[s] = a

                for bi in range(nb):
                    b = b0 + bi
                    caggs = {}
                    for s in ("lp", "hp"):
                        T_lo, T_hi = set_T[s]
                        T = T_lo + T_hi
                        aggT = psagg.tile([128, 128], f32, tag=f"aggT_{s}")
                        for t in range(T):
                            if t < T_lo:
                                gsl = gtiles[(s, "lo")][:, bi * T_lo + t, :]
                            else:
                                gsl = gtiles[(s, "hi")][:, bi * T_hi + t - T_lo, :]
                            if abuild:
                                asl_t = apool.tile([128, 128], bf16, tag="A")
                                sl = b * T + t
                                nc.vector.tensor_scalar(
                                    asl_t[:],
                                    iota[:],
                                    consts[s]["rr"][:, sl : sl + 1],
                                    consts[s]["val"][:, sl : sl + 1],
                                    mybir.AluOpType.is_equal,
                                    mybir.AluOpType.mult,
                                )
                                asl = asl_t[:]
                            else:
                                asl = atiles[s][
                                    :, (bi * T + t) * 128 : (bi * T + t + 1) * 128
                                ]
                            nc.tensor.matmul(
                                aggT[:],
                                gsl,
                                asl,
                                start=(t == 0),
                                stop=(t == T - 1),
                            )
                        cagg = caggpool.tile([128, 128], f32, tag=f"cagg_{s}")
                        nc.scalar.copy(cagg[:], aggT[:])
                        caggs[s] = cagg

                    psum2 = ps2.tile([128, 128], f32, tag="psum2")
                    nc.tensor.matmul(psum2[:], wlpT[:], caggs["lp"][:],
                                     start=True, stop=False)
                    nc.tensor.matmul(psum2[:], whpT[:], caggs["hp"][:],
                                     start=False, stop=True)
                    osb = opool.tile([128, 128], f32, tag="osb")
                    nc.scalar.activation(
                        osb[:], psum2[:], mybir.ActivationFunctionType.Relu,
                        bias=bias[:, 0:1],
                    )
                    nc.scalar.dma_start(out_t[b, :, :], osb[:])

    nc.compile()
    return nc


def kernel(x, lp_rows, lp_cols, lp_vals, hp_rows, hp_cols, hp_vals,
           W_LP, W_HP, bias, alpha_raw):
    import ml_dtypes
    from concourse.bass_utils import run_bass_kernel_spmd

    x = np.asarray(x, dtype=np.float32)
    alpha = 1.0 / (1.0 + np.exp(-float(np.asarray(alpha_raw).reshape(-1)[0])))

    plans, nblk = _plan_blocks(lp_rows, lp_cols, hp_rows, hp_cols, PACK_CAP)
    lp = _prep_set(lp_rows, lp_cols,
                   np.asarray(lp_vals, np.float32) * np.float32(alpha),
                   plans, nblk)
    hp = _prep_set(hp_rows, hp_cols,
                   np.asarray(hp_vals, np.float32) * np.float32(1.0 - alpha),
                   plans, nblk)
    idx_lo_lp, idx_hi_lp, amat_lp, rr_lp, val_lp, T_lp_lo, T_lp_hi = lp
    idx_lo_hp, idx_hi_hp, amat_hp, rr_hp, val_hp, T_hp_lo, T_hp_hi = hp
    abuild = os.environ.get("KERNEL_ABUILD", "stream") == "dve"
    key = (nblk, T_lp_lo, T_lp_hi, T_hp_lo, T_hp_hi, abuild)
    if key not in _COMPILED:
        _COMPILED[key] = _build(*key)
    nc = _COMPILED[key]

    bf = ml_dtypes.bfloat16
    xbf = np.ascontiguousarray(x.astype(bf))
    wlpT = np.ascontiguousarray(np.asarray(W_LP, np.float32).T)  # [d, o]
    whpT = np.ascontiguousarray(np.asarray(W_HP, np.float32).T)
    bias_col = np.ascontiguousarray(np.asarray(bias, np.float32).reshape(128, 1))

    iota_np = np.ascontiguousarray(
        np.tile(np.arange(128, dtype=np.float32)[None, :], (128, 1)).astype(bf))
    in_maps = []
    for c in range(NCORES):
        m = {
            "xbf": xbf,
            "idx_lo_lp": idx_lo_lp[c], "idx_hi_lp": idx_hi_lp[c],
            "idx_lo_hp": idx_lo_hp[c], "idx_hi_hp": idx_hi_hp[c],
            "wlpT": wlpT, "whpT": whpT, "bias": bias_col,
        }
        if abuild:
            m["iota"] = iota_np
            m["rr_lp"] = rr_lp[c]
            m["val_lp"] = val_lp[c]
            m["rr_hp"] = rr_hp[c]
            m["val_hp"] = val_hp[c]
        else:
            m["amat_lp"] = amat_lp[c].astype(bf)
            m["amat_hp"] = amat_hp[c].astype(bf)
        in_maps.append(m)

    trace = bool(int(os.environ.get("KERNEL_TRACE", "0")))
    res = run_bass_kernel_spmd(nc, in_maps, list(range(NCORES)), trace=trace)
    kernel.last_result = res

    out = np.empty((N_NODES, D), dtype=np.float32)
    for c in range(NCORES):
        oc = res.results[c]["out"]  # [nblk, 128o, 128r]
        base = c * ROWS_PER_CORE
        for b, (r0, r1) in enumerate(plans[c]):
            out[base + r0 : base + r1, :] = oc[b, :, : r1 - r0].T
    return out
